# revision 6
# baseline (speedup 1.0000x reference)
"""MoE layer (8 experts, top-2, shared expert) on 8 TRN2 NeuronCores.

Sparse expert-parallel: core e holds expert e's weights. Each core routes
on-device: fp32 router for ALL tokens (top-2 selection needs true fp32 -
the smallest top2/top3 prob gap in-distribution is ~3e-6), gpsimd
sparse_gather compacts the token ids routed to ITS expert (capacity
C=2560 vs ~2048 expected), a transposing dma_gather pulls those tokens'
bf16 activations, and the expert MLP runs in bf16 on just those tokens
(~4x fewer expert FLOPs than dense). The routing weight w is packed into
the fractional part of the compacted value (val = t + w/4 + 0.125) and
recovered on device via an int-cast roundtrip.

On this stack every matmul pays a serial ~107ns LDWEIGHTS unless
consecutive matmuls in the FINAL PE stream share the stationary operand
(the legalizer then emits one Ldweights). The kernel is therefore
organized as homogeneous phase sweeps so the scheduler has no foreign PE
work to interleave into the dedup chains:
  P0  router sweep over fp32 hT (self-loading fp32 matmuls) + compaction
  P1  shared gate/up sweep over bf16 hbfT in chunk PAIRS (one weight
      load feeds both chunks), staging A_sh for all T tokens in SBUF
  P2  shared down sweep, dc-paired so each stationary A-tile loads once
      per dc-pair; sigmoid gating via Copy-with-scale on ACT/DVE
  P3  dma_gather + expert MLP per 512-token chunk, dc-paired down
The ACT engine only ever needs the Exp and Silu tables (sigmoid is
computed in exp-form; Copy needs no table).

Outputs per core: o_sh (dense shared partial [T,D]), o_ex (compact
weighted expert rows [C,D]), vals (packed ids+weights), nf (count). Host
sums the 8 shared partials and fancy-index-adds each core's expert rows.
"""
import numpy as np
import ml_dtypes

T, D, E, F, FS = 8192, 2048, 8, 1024, 2048
FSS = FS // 8          # per-core shared-expert slice
NCORES = 8
C1 = 512               # sweep chunk
NCH = T // C1          # 16
NPR = NCH // 2         # 8 chunk pairs
DT = D // 128          # 16 contraction tiles
FT = F // 128          # 8 expert f-tiles
ST = FSS // 128        # 2 shared f-tiles
TJ = T // 128          # 64 token subtiles (dense)
CAP = 2560             # expert token capacity (observed max ~2100)
GC = CAP // 512        # 5 gather chunks
DC = D // 512          # 4 output column chunks

_CACHE = {}


def _build(wrap_loop=True):
    import contextlib
    import concourse.mybir as mybir
    import concourse.tile as tile
    from concourse import bacc

    F32 = mybir.dt.float32
    F32R = mybir.dt.float32r
    BF16 = mybir.dt.bfloat16
    I16 = mybir.dt.int16
    I32 = mybir.dt.int32
    U32 = mybir.dt.uint32
    AF = mybir.ActivationFunctionType
    ALU = mybir.AluOpType
    AX = mybir.AxisListType

    nc = bacc.Bacc("TRN2", target_bir_lowering=False, debug=False,
                   num_devices=NCORES)
    hT = nc.dram_tensor("hT", [D, T], F32, kind="ExternalInput").ap()
    hbfT = nc.dram_tensor("hbfT", [D, T], BF16, kind="ExternalInput").ap()
    hbf = nc.dram_tensor("hbf", [T, D], BF16, kind="ExternalInput").ap()
    gw9 = nc.dram_tensor("gw9", [D, 9], F32, kind="ExternalInput").ap()
    iota = nc.dram_tensor("iota", [128, TJ], F32, kind="ExternalInput").ap()
    esel = nc.dram_tensor("esel", [128, 8], F32, kind="ExternalInput").ap()
    wgb = nc.dram_tensor("wgb", [D, F], BF16, kind="ExternalInput").ap()
    wub = nc.dram_tensor("wub", [D, F], BF16, kind="ExternalInput").ap()
    wdb = nc.dram_tensor("wdb", [F, D], BF16, kind="ExternalInput").ap()
    wsgb = nc.dram_tensor("wsgb", [D, FSS], BF16, kind="ExternalInput").ap()
    wsub = nc.dram_tensor("wsub", [D, FSS], BF16, kind="ExternalInput").ap()
    wsdb = nc.dram_tensor("wsdb", [FSS, D], BF16, kind="ExternalInput").ap()
    nreps = nc.dram_tensor("nreps", [1, 1], U32, kind="ExternalInput").ap()
    o_sh = nc.dram_tensor("o_sh", [T, D], F32, kind="ExternalOutput").ap()
    o_ex = nc.dram_tensor("o_ex", [CAP, D], F32, kind="ExternalOutput").ap()
    vals = nc.dram_tensor("vals", [CAP, 1], F32, kind="ExternalOutput").ap()
    nf = nc.dram_tensor("nf", [1, 1], U32, kind="ExternalOutput").ap()

    def re(ap):  # [(a p), n] -> [p, a, n] DRAM view for SBUF d-tile layout
        return ap.rearrange("(a p) n -> p a n", p=128)

    def router(ps_l, rtr, gwt, eselt, hTt, comb_sb, sig_sb, c):
        """Fp32 router for one C1-chunk; ps_l holds C1//128 subtiles x 9."""
        for tsub in range(C1 // 128):
            j = c * (C1 // 128) + tsub
            sl = slice(tsub * 128, (tsub + 1) * 128)
            psl = ps_l[:, tsub * 9:(tsub + 1) * 9]
            for k in range(DT):
                nc.tensor.matmul(psl, hTt[:, k, sl].bitcast(F32),
                                 gwt[:, k, :], start=(k == 0),
                                 stop=(k == DT - 1))
            lg = rtr.tile([128, 9], F32, name="lg", tag="lg")
            nc.vector.tensor_copy(lg[:], psl)
            m1 = rtr.tile([128, 1], F32, name="m1", tag="m1")
            nc.vector.tensor_reduce(m1[:], lg[:, 0:8], axis=AX.X, op=ALU.max)
            mask1 = rtr.tile([128, 8], F32, name="mask1", tag="mask1")
            nc.vector.tensor_scalar(mask1[:], lg[:, 0:8], m1[:], None,
                                    op0=ALU.is_ge)
            lm = rtr.tile([128, 8], F32, name="lm", tag="lm")
            nc.vector.scalar_tensor_tensor(lm[:], mask1[:], -1e30, lg[:, 0:8],
                                           op0=ALU.mult, op1=ALU.add)
            m2 = rtr.tile([128, 1], F32, name="m2", tag="m2")
            nc.vector.tensor_reduce(m2[:], lm[:], axis=AX.X, op=ALU.max)
            mask2 = rtr.tile([128, 8], F32, name="mask2", tag="mask2")
            nc.vector.tensor_scalar(mask2[:], lm[:], m2[:], None, op0=ALU.is_ge)
            nm1 = rtr.tile([128, 1], F32, name="nm1", tag="nm1")
            nc.vector.tensor_scalar(nm1[:], m1[:], -1.0, None, op0=ALU.mult)
            ex = rtr.tile([128, 8], F32, name="ex", tag="ex")
            nc.scalar.activation(ex[:], lg[:, 0:8], AF.Exp, bias=nm1[:],
                                 scale=1.0)
            m12 = rtr.tile([128, 8], F32, name="m12", tag="m12")
            nc.vector.tensor_tensor(m12[:], mask1[:], mask2[:], op=ALU.add)
            em = rtr.tile([128, 8], F32, name="em", tag="em")
            nc.vector.tensor_tensor(em[:], ex[:], m12[:], op=ALU.mult)
            den = rtr.tile([128, 1], F32, name="den", tag="den")
            nc.vector.tensor_reduce(den[:], em[:], axis=AX.X, op=ALU.add)
            rden = rtr.tile([128, 1], F32, name="rden", tag="rden")
            nc.vector.reciprocal(rden[:], den[:])
            comb9 = rtr.tile([128, 8], F32, name="comb9", tag="comb9")
            nc.vector.tensor_scalar(comb9[:], em[:], rden[:], None,
                                    op0=ALU.mult)
            ce = rtr.tile([128, 8], F32, name="ce", tag="ce")
            nc.vector.tensor_tensor(ce[:], comb9[:], eselt[:], op=ALU.mult)
            nc.vector.tensor_reduce(comb_sb[:, j:j + 1], ce[:], axis=AX.X,
                                    op=ALU.add)
            # sigmoid in exp-form (keeps ACT on the Exp table): 1/(1+e^-x)
            es = rtr.tile([128, 1], F32, name="es", tag="es")
            nc.scalar.activation(es[:], lg[:, 8:9], AF.Exp, scale=-1.0)
            es1 = rtr.tile([128, 1], F32, name="es1", tag="es1")
            nc.vector.tensor_scalar(es1[:], es[:], 1.0, None, op0=ALU.add)
            nc.vector.reciprocal(sig_sb[:, j:j + 1], es1[:])

    with tile.TileContext(nc) as tc:
        if wrap_loop:
            tmp = nc.alloc_registers("tmp_nreps", mybir.ALL_ENGINES)
            nc.regs_load(tmp, nreps[0:1, 0:1])
            rv = nc.snap(tmp, donate=True, min_val=1, max_val=4096)
            loop_cm = tc.For_i(0, rv, 1)
        else:
            loop_cm = contextlib.nullcontext()
        with loop_cm:
            with tc.tile_pool(name="pers", bufs=1) as pers:
                gwt = pers.tile([128, DT, 9], F32, name="gwt")
                nc.sync.dma_start(out=gwt[:], in_=re(gw9))
                eselt = pers.tile([128, 8], F32, name="eselt")
                nc.sync.dma_start(out=eselt[:], in_=esel)
                iotat = pers.tile([128, TJ], F32, name="iotat")
                nc.sync.dma_start(out=iotat[:], in_=iota)
                comb_sb = pers.tile([128, TJ], F32, name="comb_sb")
                sig_sb = pers.tile([128, TJ], F32, name="sig_sb")
                w128 = pers.tile([128, CAP // 128], F32, name="w128")
                idxr = pers.tile([128, CAP // 16], I16, name="idxr")

                # a_sh_all: shared-expert activations for all T tokens
                with tc.tile_pool(name="w1", bufs=1) as w1, \
                     tc.tile_pool(name="ash", bufs=1) as ash, \
                     tc.tile_pool(name="cp", bufs=1) as cp:
                    wsgt = w1.tile([128, DT, FSS], BF16, name="wsgt")
                    nc.sync.dma_start(out=wsgt[:], in_=re(wsgb))
                    wsut = w1.tile([128, DT, FSS], BF16, name="wsut")
                    nc.sync.dma_start(out=wsut[:], in_=re(wsub))
                    wsdt = w1.tile([128, ST, D], BF16, name="wsdt")
                    nc.sync.dma_start(out=wsdt[:], in_=re(wsdb))
                    a_sh = ash.tile([128, ST, T], BF16, name="a_sh")

                    # ---- P0: router sweep (fp32) ----
                    with tc.tile_pool(name="h0", bufs=2) as h0, \
                         tc.tile_pool(name="rtr", bufs=2) as rtr, \
                         tc.tile_pool(name="psr", bufs=2, space="PSUM") as psr:
                        for c in range(NCH):
                            t0 = c * C1
                            hTt = h0.tile([128, DT, C1], F32R, name="hTt",
                                          tag="hTt")
                            nc.sync.dma_start(
                                out=hTt[:],
                                in_=re(hT[:, t0:t0 + C1]).bitcast(F32R))
                            ps_l = psr.tile([128, (C1 // 128) * 9], F32,
                                            name="ps_l", tag="ps_l")
                            router(ps_l[:], rtr, gwt, eselt, hTt, comb_sb,
                                   sig_sb, c)

                        # ---- compaction (gpsimd/DVE; overlaps P1) ----
                        va = cp.tile([128, TJ], F32, name="va")
                        nc.vector.tensor_scalar(va[:], comb_sb[:], 0.25, 0.125,
                                                op0=ALU.mult, op1=ALU.add)
                        nc.vector.tensor_tensor(va[:], va[:], iotat[:],
                                                op=ALU.add)
                        vm = cp.tile([128, TJ], F32, name="vm")
                        nc.vector.tensor_scalar(vm[:], comb_sb[:], 0.0, None,
                                                op0=ALU.is_gt)
                        nc.vector.tensor_scalar(va[:], va[:], 1.0, None,
                                                op0=ALU.add)
                        nc.vector.tensor_tensor(va[:], vm[:], va[:],
                                                op=ALU.mult)
                        nc.vector.tensor_scalar(va[:], va[:], -1.0, None,
                                                op0=ALU.add)
                        v16 = cp.tile([16, TJ, 8], F32, name="v16")
                        for r in range(8):
                            nc.sync.dma_start(out=v16[:, :, r],
                                              in_=va[16 * r:16 * (r + 1), :])
                        pk = cp.tile([16, CAP // 16], F32, name="pk")
                        nft = cp.tile([1, 1], U32, name="nft")
                        nc.gpsimd.sparse_gather(pk[:], v16[:],
                                                num_found=nft[:])
                        nc.sync.dma_start(out=nf, in_=nft[:])
                        nc.sync.dma_start(
                            out=vals.rearrange("(f q) o -> q (f o)", q=16),
                            in_=pk[:])
                        pi = cp.tile([16, CAP // 16], I32, name="pi")
                        nc.vector.tensor_copy(pi[:], pk[:])
                        nc.vector.tensor_scalar(pi[:], pi[:], T - 1, 0,
                                                op0=ALU.min, op1=ALU.max)
                        idx16 = cp.tile([16, CAP // 16], I16, name="idx16")
                        nc.vector.tensor_copy(idx16[:], pi[:])
                        for r in range(8):
                            nc.sync.dma_start(
                                out=idxr[16 * r:16 * (r + 1), :], in_=idx16[:])
                        vg = cp.tile([128, CAP // 128], F32, name="vg")
                        nc.sync.dma_start(
                            out=vg[:],
                            in_=vals.rearrange("(k p) o -> p (k o)", p=128))
                        vgi = cp.tile([128, CAP // 128], I32, name="vgi")
                        nc.vector.tensor_copy(vgi[:], vg[:])
                        nc.vector.tensor_scalar(vgi[:], vgi[:], T - 1, 0,
                                                op0=ALU.min, op1=ALU.max)
                        vgf = cp.tile([128, CAP // 128], F32, name="vgf")
                        nc.vector.tensor_copy(vgf[:], vgi[:])
                        nc.vector.tensor_tensor(w128[:], vg[:], vgf[:],
                                                op=ALU.subtract)
                        nc.vector.tensor_scalar(w128[:], w128[:], 4.0, -0.5,
                                                op0=ALU.mult, op1=ALU.add)

                    # ---- P1: shared gate/up sweep (bf16, chunk pairs) ----
                    with tc.tile_pool(name="hb", bufs=4) as hb, \
                         tc.tile_pool(name="rt1", bufs=3) as rt1, \
                         tc.tile_pool(name="psg", bufs=1, space="PSUM") as psg:
                        for pr in range(NPR):
                            hbts = []
                            for ci in range(2):
                                t0 = (2 * pr + ci) * C1
                                hbt = hb.tile([128, DT, C1], BF16, name="hbt",
                                              tag="hbt")
                                nc.sync.dma_start(
                                    out=hbt[:], in_=re(hbfT[:, t0:t0 + C1]))
                                hbts.append(hbt)
                            for ft in range(ST):
                                off = ft * 128
                                # [a.g | a.u | b.g | b.u] 512 cols each
                                ps_p = psg.tile([128, 2048], F32, name="ps_p",
                                                tag="ps_p")
                                for hf, wt in ((0, wsgt), (1, wsut)):
                                    for k in range(DT):
                                        for ci in range(2):
                                            o0 = ci * 1024 + hf * C1
                                            nc.tensor.matmul(
                                                ps_p[:, o0:o0 + C1],
                                                wt[:, k, off:off + 128],
                                                hbts[ci][:, k, :],
                                                start=(k == 0),
                                                stop=(k == DT - 1))
                                for ci in range(2):
                                    o0 = ci * 1024
                                    tsl = slice((2 * pr + ci) * C1,
                                                (2 * pr + ci + 1) * C1)
                                    sg = rt1.tile([128, C1], F32, name="sg",
                                                  tag="sg")
                                    nc.scalar.activation(
                                        sg[:], ps_p[:, o0:o0 + C1], AF.Silu)
                                    nc.vector.tensor_tensor(
                                        a_sh[:, ft, tsl], sg[:],
                                        ps_p[:, o0 + C1:o0 + 2 * C1],
                                        op=ALU.mult)

                    # ---- P2: shared down sweep ----
                    with tc.tile_pool(name="ot2", bufs=4) as ot2, \
                         tc.tile_pool(name="psd", bufs=2, space="PSUM") as psd:
                        for j in range(TJ):
                            tsl = slice(j * 128, (j + 1) * 128)
                            for dcp in range(2):
                                ps_s = psd.tile([128, 2, 512], F32,
                                                name="ps_s", tag="ps_s")
                                for ft in range(ST):
                                    for dh in range(2):
                                        dci = dcp * 2 + dh
                                        dsl = slice(dci * 512,
                                                    (dci + 1) * 512)
                                        nc.tensor.matmul(
                                            ps_s[:, dh, :], a_sh[:, ft, tsl],
                                            wsdt[:, ft, dsl],
                                            start=(ft == 0),
                                            stop=(ft == ST - 1))
                                for dh in range(2):
                                    dci = dcp * 2 + dh
                                    dsl = slice(dci * 512, (dci + 1) * 512)
                                    ot = ot2.tile([128, 512], F32, name="ot",
                                                  tag="ot")
                                    if dh == 0:
                                        nc.scalar.activation(
                                            ot[:], ps_s[:, dh, :], AF.Copy,
                                            scale=sig_sb[:, j:j + 1])
                                    else:
                                        nc.vector.tensor_scalar(
                                            ot[:], ps_s[:, dh, :],
                                            sig_sb[:, j:j + 1], None,
                                            op0=ALU.mult)
                                    nc.sync.dma_start(
                                        out=o_sh[j * 128:(j + 1) * 128, dsl],
                                        in_=ot[:])

                # ---- P3: gather routed tokens, expert MLP ----
                with tc.tile_pool(name="w3", bufs=1) as w3, \
                     tc.tile_pool(name="h3", bufs=2) as h3, \
                     tc.tile_pool(name="ag", bufs=2) as ag, \
                     tc.tile_pool(name="o3", bufs=4) as o3, \
                     tc.tile_pool(name="rt3", bufs=2) as rt3, \
                     tc.tile_pool(name="ps3", bufs=2, space="PSUM") as ps3, \
                     tc.tile_pool(name="ps3y", bufs=2, space="PSUM") as ps3y:
                    wgt = w3.tile([128, DT, F], BF16, name="wgt")
                    nc.sync.dma_start(out=wgt[:], in_=re(wgb))
                    wut = w3.tile([128, DT, F], BF16, name="wut")
                    nc.sync.dma_start(out=wut[:], in_=re(wub))
                    wdt = w3.tile([128, FT, D], BF16, name="wdt")
                    nc.sync.dma_start(out=wdt[:], in_=re(wdb))

                    for gc in range(GC):
                        hTg = h3.tile([128, DT, 512], BF16, name="hTg",
                                      tag="hTg")
                        nc.gpsimd.dma_gather(hTg[:], hbf,
                                             idxr[:, gc * 32:(gc + 1) * 32],
                                             512, 512, D, transpose=True)
                        a_g = ag.tile([128, FT, 512], BF16, name="a_g",
                                      tag="a_g")
                        for ft in range(FT):
                            off = ft * 128
                            ps_p = ps3.tile([128, 1024], F32, name="ps_p3",
                                            tag="ps_p3")
                            for k in range(DT):
                                nc.tensor.matmul(ps_p[:, 0:512],
                                                 wgt[:, k, off:off + 128],
                                                 hTg[:, k, :], start=(k == 0),
                                                 stop=(k == DT - 1))
                            for k in range(DT):
                                nc.tensor.matmul(ps_p[:, 512:1024],
                                                 wut[:, k, off:off + 128],
                                                 hTg[:, k, :], start=(k == 0),
                                                 stop=(k == DT - 1))
                            sg = rt3.tile([128, 512], F32, name="sg3",
                                          tag="sg3")
                            nc.scalar.activation(sg[:], ps_p[:, 0:512],
                                                 AF.Silu)
                            nc.vector.tensor_tensor(a_g[:, ft, :], sg[:],
                                                    ps_p[:, 512:1024],
                                                    op=ALU.mult)
                        for tsub in range(4):
                            kcol = gc * 4 + tsub
                            tsl = slice(tsub * 128, (tsub + 1) * 128)
                            r0 = gc * 512 + tsub * 128
                            for dcp in range(2):
                                ps_y = ps3y.tile([128, 2, 512], F32,
                                                 name="ps_y", tag="ps_y")
                                for ft in range(FT):
                                    for dh in range(2):
                                        dci = dcp * 2 + dh
                                        dsl = slice(dci * 512,
                                                    (dci + 1) * 512)
                                        nc.tensor.matmul(
                                            ps_y[:, dh, :], a_g[:, ft, tsl],
                                            wdt[:, ft, dsl],
                                            start=(ft == 0),
                                            stop=(ft == FT - 1))
                                for dh in range(2):
                                    dci = dcp * 2 + dh
                                    dsl = slice(dci * 512, (dci + 1) * 512)
                                    ot = o3.tile([128, 512], F32, name="ot3",
                                                 tag="ot3")
                                    if dh == 0:
                                        nc.scalar.activation(
                                            ot[:], ps_y[:, dh, :], AF.Copy,
                                            scale=w128[:, kcol:kcol + 1])
                                    else:
                                        nc.vector.tensor_scalar(
                                            ot[:], ps_y[:, dh, :],
                                            w128[:, kcol:kcol + 1], None,
                                            op0=ALU.mult)
                                    nc.sync.dma_start(
                                        out=o_ex[r0:r0 + 128, dsl], in_=ot[:])
    nc.compile()
    return nc


def _get_nc():
    if "nc" not in _CACHE:
        _CACHE["nc"] = _build()
    return _CACHE["nc"]


def _in_maps(inputs, nreps=1):
    h = np.ascontiguousarray(inputs["hidden_states"], dtype=np.float32)
    hT = np.ascontiguousarray(h.T)
    hbf = np.ascontiguousarray(h.astype(ml_dtypes.bfloat16))
    hbfT = np.ascontiguousarray(hT.astype(ml_dtypes.bfloat16))
    gw9 = np.ascontiguousarray(
        np.concatenate([inputs["gate_w"], inputs["wsg"]], axis=1),
        dtype=np.float32)
    iota = (np.arange(TJ)[None, :] * 128
            + np.arange(128)[:, None]).astype(np.float32)
    nr = np.array([[nreps]], dtype=np.uint32)
    bf = lambda a: np.ascontiguousarray(np.asarray(a, np.float32)
                                        .astype(ml_dtypes.bfloat16))
    maps = []
    for e in range(NCORES):
        es = np.zeros((128, 8), np.float32)
        es[:, e] = 1.0
        maps.append({
            "hT": hT,
            "hbfT": hbfT,
            "hbf": hbf,
            "gw9": gw9,
            "iota": iota,
            "esel": es,
            "wgb": bf(inputs["w_gate"][e]),
            "wub": bf(inputs["w_up"][e]),
            "wdb": bf(inputs["w_down"][e]),
            "wsgb": bf(inputs["ws_gate"][:, e * FSS:(e + 1) * FSS]),
            "wsub": bf(inputs["ws_up"][:, e * FSS:(e + 1) * FSS]),
            "wsdb": bf(inputs["ws_down"][e * FSS:(e + 1) * FSS, :]),
            "nreps": nr,
        })
    return maps


def _run(inputs, nreps=1):
    from concourse.bass_utils import run_bass_kernel_spmd
    nc = _get_nc()
    res = run_bass_kernel_spmd(nc, _in_maps(inputs, nreps),
                               core_ids=list(range(NCORES)))
    return res


def kernel(**inputs):
    res = _run(inputs, nreps=1)
    out = res.results[0]["o_sh"].astype(np.float32).copy()
    for e in range(1, NCORES):
        out += res.results[e]["o_sh"]
    for e in range(NCORES):
        r = res.results[e]
        n = min(int(r["nf"][0, 0]), CAP)
        v = r["vals"][:n, 0]
        idx = np.floor(v).astype(np.int64)
        out[idx] += r["o_ex"][:n]
    return out
